# revision 6
# baseline (speedup 1.0000x reference)
"""Fused multi-head attention (QKV proj + RoPE + causal softmax + out proj)
for Trainium2, sharded over 8 NeuronCores.

Sharding: data-parallel over batch (B=2) x tensor-parallel over heads
(16 heads -> 4 per core).  Each core computes, for its (batch, head-group):
  qT/kT = wq/wk^T-projections in [d, s] layout (CDT matmuls, fp32 PSUM)
  RoPE applied on-chip (DVE pair-swap via stream_shuffle + mul/add)
  scoresT[kp, q] = krot^T.T @ qrot (one K=128 matmul per tile)
  causal masking via a PE-accumulated triangular constant on diagonal tiles
  expT = exp(scale * scoresT) on ACT, software-pipelined one kt-tile ahead
  PV with a ones-augmented V column => unnormalized out + softmax denominator
  normalize (DVE reciprocal + tensor_scalar), PE-transpose to attnT[d, s]
  partial output y_g = attnT.T @ wo_rows  (summed over head-groups on host)

Scheduling notes (v2):
  - weights (wq/wk/wv) are SBUF-resident, streamed once in kc-chunks; x is
    loaded once per s-block and reused by all three projections.
  - the attention inner loop issues scores(kt)/exp(kt) one step ahead of
    PV(kt-1) so the PE never waits on the ACT engine.
  - per-head normalize runs inline; the PE transposes of head h and the
    out-projection tiles of q-block qb-1 are deferred "units" drip-fed into
    later kt steps, filling PE time while ACT catches up.

Inputs arrive full-size; host slices/transposes, feeds 8 SPMD cores, and
sums the 4 head-group partials per batch at the end.
"""

import math

import numpy as np

import concourse.bacc as bacc
import concourse.mybir as mybir
from concourse import tile
from concourse.bass_utils import run_bass_kernel_spmd

B, S, D, H = 2, 2048, 2048, 16
NCORES = 8
HG = 4  # heads per core
HD = D // H  # 128
DG = HG * HD  # 512 = per-core slice of D
P = 128
NKC = D // P  # 16 contraction chunks
SBLK = 512  # s-block width in projection passes
NSB = S // SBLK
NST = S // P  # 16 s-tiles of 128
QB = 512  # q-block width in attention
NQB = S // QB
NQT = QB // P  # q-subtiles per block
EB = 512  # e-block width in out-projection
NEB = D // EB

F32 = mybir.dt.float32
EXP = mybir.ActivationFunctionType.Exp
SCALE = 1.0 / math.sqrt(HD)
SWAP32 = [i ^ 1 for i in range(32)]
NEG = -1.0e9

COMPUTE_DTYPE = "bfloat16"


def build_program(variant: str, dump: bool = False, cdt_name: str | None = None):
    """variant: 'causal' | 'none' | 'general'"""
    CDT = getattr(mybir.dt, cdt_name or COMPUTE_DTYPE)
    nc = bacc.Bacc("TRN2", target_bir_lowering=False, debug=False)
    xT = nc.dram_tensor("xT", [D, S], CDT, kind="ExternalInput")
    wq = nc.dram_tensor("wq", [D, DG], CDT, kind="ExternalInput")
    wk = nc.dram_tensor("wk", [D, DG], CDT, kind="ExternalInput")
    wv = nc.dram_tensor("wv", [D, DG], CDT, kind="ExternalInput")
    wo = nc.dram_tensor("wo", [DG, D], CDT, kind="ExternalInput")
    cosT = nc.dram_tensor("cosT", [HD, S], CDT, kind="ExternalInput")
    sinT = nc.dram_tensor("sinT", [HD, S], CDT, kind="ExternalInput")
    ident = nc.dram_tensor("ident", [P, P], CDT, kind="ExternalInput")
    tri = None
    maskT = None
    if variant == "causal":
        tri = nc.dram_tensor("tri", [P, P], CDT, kind="ExternalInput")
    elif variant == "general":
        # mask.T pre-scaled by sqrt(HD) on host so exp's scale recovers it
        maskT = nc.dram_tensor("maskT", [S, S], CDT, kind="ExternalInput")
    y = nc.dram_tensor("y", [S, D], F32, kind="ExternalOutput")
    d_qrot = d_krot = d_vaug = d_attnT = None
    if dump:
        d_qrot = nc.dram_tensor("d_qrot", [P, HG, S], CDT, kind="ExternalOutput")
        d_krot = nc.dram_tensor("d_krot", [P, HG, S], CDT, kind="ExternalOutput")
        d_vaug = nc.dram_tensor("d_vaug", [P, NST, HG, HD + 2], CDT, kind="ExternalOutput")
        d_attnT = nc.dram_tensor("d_attnT", [P, HG, S], CDT, kind="ExternalOutput")

    with tile.TileContext(nc) as tc:
        with (
            tc.tile_pool(name="const", bufs=1) as constp,
            tc.tile_pool(name="big", bufs=1) as bigp,
            # 6 PSUM banks cycled between proj accumulators / scores / PV-aug
            tc.tile_pool(name="ps", bufs=1, space="PSUM") as pspool,
            # 2 PSUM banks shared by transposes + out-projection
            tc.tile_pool(name="psaux", bufs=2, space="PSUM") as psaux,
        ):
            # --- constants on the vector DMA queue (not startup-critical) ---
            tid = constp.tile([P, P], CDT)
            nc.sync.dma_start(tid[:], ident[:])
            ttri = None
            if variant == "causal":
                ttri = constp.tile([P, P], CDT)
                nc.sync.dma_start(ttri[:], tri[:])

            # --- resident weights in kc chunks on the scalar queue, ordered by
            # first-use time: wq, wk, cos/sin (RoPE of the first q drain), wv.
            # Fine granularity lets the first matmul start ~1.5us after DMA
            # comes up instead of waiting on MB-sized chunks. ---
            wres = {}
            for pname in ("q", "k", "v"):
                wres[pname] = bigp.tile([P, NKC, DG], CDT, tag=f"w{pname}", name=f"w{pname}")

            def stream_w(pname, wdram):
                for kc in range(NKC):
                    nc.scalar.dma_start(
                        wres[pname][:, kc, :],
                        wdram[kc * P : (kc + 1) * P, :],
                    )

            stream_w("q", wq)
            stream_w("k", wk)
            tcos = constp.tile([HD, S], CDT)
            nc.scalar.dma_start(tcos[:], cosT[:])
            tsin = constp.tile([HD, S], CDT)
            nc.scalar.dma_start(tsin[:], sinT[:])
            stream_w("v", wv)
            # wo rides the gpsimd queue behind the sb0/sb1 x chunks (first
            # needed by the qb=1 out-projection, far into the attention phase)
            wo_sb = bigp.tile([P, HG, D], CDT, tag="wo")

            qrot = bigp.tile([P, HG, S], CDT, tag="qrot")
            krot = bigp.tile([P, HG, S], CDT, tag="krot")
            vaug = bigp.tile([P, NST, HG, HD + 2], CDT, tag="vaug")
            ones_view = vaug[:, :, :, HD : HD + 2]
            nc.vector.memset(ones_view, 1.0)
            attnT = bigp.tile([P, HG, S], CDT, tag="attnT")

            # PSUM ring: 4 'aug' + 2 'sc' slots = 6 banks, cycled by the
            # projection passes; the attention phase uses 'sc' as the
            # scores double-buffer and 'aug' for the 4 PV accumulators.
            ring = ["aug", "aug", "aug", "aug", "sc", "sc"]
            ring_bufs = {"aug": 4, "sc": 2}
            ring_pos = [0]

            def acc_tile(width, name):
                tag = ring[ring_pos[0] % 6]
                ring_pos[0] += 1
                return pspool.tile(
                    [P, width], F32, tag=tag, bufs=ring_bufs[tag], name=name
                )

            # ---------------- projections + RoPE ----------------
            with (
                tc.tile_pool(name="xpool", bufs=2) as xpool,
                tc.tile_pool(name="rope", bufs=3) as ropep,
            ):
                for sb in range(NSB):
                    # x rows for this s-block, loaded once, reused by q/k/v
                    xt = xpool.tile([P, NKC, SBLK], CDT, tag="xt", name="xt")
                    for kc in range(NKC):
                        nc.gpsimd.dma_start(
                            xt[:, kc, :],
                            xT[
                                kc * P : (kc + 1) * P,
                                sb * SBLK : (sb + 1) * SBLK,
                            ],
                        )
                    if sb == 1:
                        nc.gpsimd.dma_start(
                            wo_sb[:], wo.ap().rearrange("(dc p) e -> p dc e", p=P)
                        )
                    for proj in ("q", "k", "v"):
                        wt = wres[proj]
                        nun = SBLK // P if proj == "v" else HG
                        pss = [
                            acc_tile(
                                SBLK if proj != "v" else DG, f"ps_{proj}_{sb}_{u}"
                            )
                            for u in range(nun)
                        ]
                        for kc in range(NKC):
                            if proj in ("q", "k"):
                                for dt in range(HG):
                                    nc.tensor.matmul(
                                        pss[dt][:],
                                        wt[:, kc, dt * HD : (dt + 1) * HD],
                                        xt[:, kc, :],
                                        start=(kc == 0),
                                        stop=(kc == NKC - 1),
                                    )
                            else:
                                for st in range(SBLK // P):
                                    nc.tensor.matmul(
                                        pss[st][:],
                                        xt[:, kc, st * P : (st + 1) * P],
                                        wt[:, kc, :],
                                        start=(kc == 0),
                                        stop=(kc == NKC - 1),
                                    )
                        if proj in ("q", "k"):
                            dstbuf = qrot if proj == "q" else krot
                            ssl = slice(sb * SBLK, (sb + 1) * SBLK)
                            # drain all four PSUM banks first (fast copies),
                            # then run the RoPE chains from SBUF
                            qsbs = []
                            for dt in range(HG):
                                qsb = ropep.tile(
                                    [P, SBLK], CDT, tag=f"qsb{dt}", name="qsb"
                                )
                                nc.vector.tensor_copy(qsb[:], pss[dt][:])
                                qsbs.append(qsb)
                            for dt in range(HG):
                                qsb = qsbs[dt]
                                tsw = ropep.tile([P, SBLK], CDT, tag="tsw", name="tsw")
                                nc.vector.stream_shuffle(tsw[:], qsb[:], SWAP32)
                                t1 = ropep.tile([P, SBLK], CDT, tag="t1", name="t1")
                                nc.vector.tensor_mul(t1[:], qsb[:], tcos[:, ssl])
                                t2 = ropep.tile([P, SBLK], CDT, tag="t2", name="t2")
                                nc.vector.tensor_mul(t2[:], tsw[:], tsin[:, ssl])
                                nc.vector.tensor_add(dstbuf[:, dt, ssl], t1[:], t2[:])
                        else:
                            for st in range(SBLK // P):
                                st_g = sb * (SBLK // P) + st
                                nc.gpsimd.tensor_copy(
                                    vaug[:, st_g, :, 0:HD],
                                    pss[st][:].rearrange("p (h d) -> p h d", d=HD),
                                )

            if dump:
                nc.sync.dma_start(d_qrot.ap(), qrot[:])
                nc.sync.dma_start(d_krot.ap(), krot[:])
                nc.sync.dma_start(d_vaug.ap(), vaug[:])

            # ---------------- attention + interleaved out-projection ----------------
            with (
                tc.tile_pool(name="mask", bufs=1) as maskp,
                tc.tile_pool(name="expp", bufs=4) as epool,
                tc.tile_pool(name="small", bufs=4) as smallp,
                tc.tile_pool(name="normp", bufs=1) as npool,
                tc.tile_pool(name="outp", bufs=4) as outp,
            ):
                # deferred PE work (transposes of the previous head,
                # out-proj tiles of the previous q-block), drip-fed into
                # the kt loop so the PE stays ahead of the ACT engine
                units = []

                def emit_units(k):
                    for _ in range(min(k, len(units))):
                        units.pop(0)()

                def transpose_unit(h, qt_g, attn_n):
                    def run():
                        ps_t = psaux.tile([P, P], CDT, tag="tr", name="tr")
                        nc.tensor.transpose(ps_t[:], attn_n[:], tid[:])
                        nc.vector.tensor_copy(
                            attnT[:, h, qt_g * P : (qt_g + 1) * P], ps_t[:]
                        )
                    return run

                def outproj_unit(st, eb):
                    def run():
                        ps_o = psaux.tile([P, EB], F32, tag="tr", name=f"o{st}_{eb}")
                        for dc in range(HG):
                            nc.tensor.matmul(
                                ps_o[:],
                                attnT[:, dc, st * P : (st + 1) * P],
                                wo_sb[:, dc, eb * EB : (eb + 1) * EB],
                                start=(dc == 0),
                                stop=(dc == HG - 1),
                            )
                        out_t = outp.tile([P, EB], F32, tag="outsb", name="outsb")
                        nc.gpsimd.tensor_copy(out_t[:], ps_o[:])
                        nc.sync.dma_start(
                            y[st * P : (st + 1) * P, eb * EB : (eb + 1) * EB],
                            out_t[:],
                        )
                    return run

                def finish_qt(h, qb, qt, aug):
                    """normalize one finished q-subtile accumulator (DVE),
                    defer its PE transpose."""
                    qt_g = qb * NQT + qt
                    aug_sb = npool.tile(
                        [P, HD + 2], F32, tag="aug_sb", name="aug_sb", bufs=4
                    )
                    nc.gpsimd.tensor_copy(aug_sb[:], aug[:])
                    rec = smallp.tile([P, 1], F32, tag="rec", name="rec")
                    nc.vector.reciprocal(rec[:], aug_sb[:, HD : HD + 1])
                    attn_n = npool.tile(
                        [P, HD], CDT, tag="attn_n", name="attn_n", bufs=10
                    )
                    nc.vector.tensor_scalar_mul(attn_n[:], aug_sb[:, 0:HD], rec[:])
                    units.append(transpose_unit(h, qt_g, attn_n))

                for qb in range(NQB):
                    mts = None
                    if variant == "general":
                        mts = maskp.tile([P, NST, QB], CDT, tag="mt", name="mt")
                        nc.sync.dma_start(
                            mts[:],
                            maskT[:, qb * QB : (qb + 1) * QB].rearrange(
                                "(kt p) q -> p kt q", p=P
                            ),
                        )
                    if qb > 0:
                        for st in range((qb - 1) * NQT, qb * NQT):
                            for eb in range(NEB):
                                units.append(outproj_unit(st, eb))
                    nkt = NQT * (qb + 1) if variant == "causal" else NST
                    for h in range(HG):
                        augs = [None] * NQT
                        exps = [None] * nkt

                        def scores_step(kt):
                            j = kt - NQT * qb  # diag index (causal)
                            ps_s = pspool.tile(
                                [P, QB], F32, tag="sc", bufs=2, name="scores"
                            )
                            if variant == "causal" and j >= 0:
                                nc.tensor.matmul(
                                    ps_s[:, j * P : QB],
                                    krot[:, h, kt * P : (kt + 1) * P],
                                    qrot[:, h, qb * QB + j * P : (qb + 1) * QB],
                                    start=True,
                                    stop=False,
                                )
                                nc.tensor.matmul(
                                    ps_s[:, j * P : (j + 1) * P],
                                    tid[:],
                                    ttri[:],
                                    start=False,
                                    stop=True,
                                )
                                valid = slice(j * P, QB)
                            else:
                                last = variant != "general"
                                nc.tensor.matmul(
                                    ps_s[:],
                                    krot[:, h, kt * P : (kt + 1) * P],
                                    qrot[:, h, qb * QB : (qb + 1) * QB],
                                    start=True,
                                    stop=last,
                                )
                                if variant == "general":
                                    nc.tensor.matmul(
                                        ps_s[:],
                                        tid[:],
                                        mts[:, kt, :],
                                        start=False,
                                        stop=True,
                                    )
                                valid = slice(0, QB)
                            texp = epool.tile([P, QB], CDT, tag="exp", name="exp")
                            nc.scalar.activation(
                                texp[:, valid], ps_s[:, valid], EXP, scale=SCALE
                            )
                            exps[kt] = texp

                        def pv_step(kt):
                            j = kt - NQT * qb
                            texp = exps[kt]
                            for qt in range(NQT):
                                if variant == "causal" and (
                                    j > qt or kt > NQT * qb + qt
                                ):
                                    continue
                                if kt == 0:
                                    augs[qt] = pspool.tile(
                                        [P, HD + 2],
                                        F32,
                                        tag="aug",
                                        bufs=4,
                                        name=f"aug{qt}",
                                    )
                                last_kt = (
                                    NQT * qb + qt if variant == "causal" else NST - 1
                                )
                                nc.tensor.matmul(
                                    augs[qt][:],
                                    texp[:, qt * P : (qt + 1) * P],
                                    vaug[:, kt, h, :],
                                    start=(kt == 0),
                                    stop=(kt == last_kt),
                                )
                                if kt == last_kt:
                                    finish_qt(h, qb, qt, augs[qt])

                        # software-pipelined: scores/exp one kt ahead of PV
                        for kt in range(nkt + 1):
                            if kt < nkt:
                                scores_step(kt)
                            emit_units(1)
                            if kt >= 1:
                                pv_step(kt - 1)

                # flush: remaining transposes + final q-block's out-projection
                emit_units(len(units))
                for st in range((NQB - 1) * NQT, NQB * NQT):
                    for eb in range(NEB):
                        outproj_unit(st, eb)()

                if dump:
                    nc.sync.dma_start(d_attnT.ap(), attnT[:])

    nc.compile()
    return nc


_PROGRAM_CACHE: dict[str, object] = {}
_last_in_maps = None


def _get_program(variant: str):
    key = f"{variant}:{COMPUTE_DTYPE}"
    if key not in _PROGRAM_CACHE:
        _PROGRAM_CACHE[key] = build_program(variant)
    return _PROGRAM_CACHE[key]


def _detect_variant(mask: np.ndarray) -> str:
    if not np.any(mask):
        return "none"
    causal = np.triu(np.full((S, S), NEG, dtype=np.float32), 1)
    if np.array_equal(mask, causal):
        return "causal"
    return "general"


def _np_cdt():
    if COMPUTE_DTYPE == "bfloat16":
        import ml_dtypes

        return ml_dtypes.bfloat16
    return np.float32


def make_in_maps(x, wq, wk, wv, wo, cos, sin, mask, variant):
    npdt = _np_cdt()
    cosT = np.repeat(cos.T, 2, axis=0)  # [HD, S]
    sinT = np.repeat(sin.T, 2, axis=0)
    sinT = sinT.copy()
    sinT[0::2, :] *= -1.0  # row 2i holds -sin, row 2i+1 holds +sin
    shared = {
        "cosT": np.ascontiguousarray(cosT).astype(npdt),
        "sinT": np.ascontiguousarray(sinT).astype(npdt),
        "ident": np.eye(P, dtype=np.float32).astype(npdt),
    }
    if variant == "causal":
        # scoresT layout is [kp, q]: masked where kp > q -> strict lower triangle
        shared["tri"] = np.tril(np.full((P, P), NEG, dtype=np.float32), -1).astype(npdt)
    elif variant == "general":
        shared["maskT"] = np.ascontiguousarray(mask.T * math.sqrt(HD)).astype(npdt)

    xTs = [np.ascontiguousarray(x[b].T).astype(npdt) for b in range(B)]
    in_maps = []
    for core in range(NCORES):
        b, g = divmod(core, NCORES // B)
        sl = slice(g * DG, (g + 1) * DG)
        in_maps.append(
            {
                "xT": xTs[b],
                "wq": np.ascontiguousarray(wq[:, sl]).astype(npdt),
                "wk": np.ascontiguousarray(wk[:, sl]).astype(npdt),
                "wv": np.ascontiguousarray(wv[:, sl]).astype(npdt),
                "wo": np.ascontiguousarray(wo[sl, :]).astype(npdt),
                **shared,
            }
        )
    return in_maps


def kernel(x, wq, wk, wv, wo, cos, sin, mask):
    x = np.asarray(x, dtype=np.float32)
    wq = np.asarray(wq, dtype=np.float32)
    wk = np.asarray(wk, dtype=np.float32)
    wv = np.asarray(wv, dtype=np.float32)
    wo = np.asarray(wo, dtype=np.float32)
    cos = np.asarray(cos, dtype=np.float32)
    sin = np.asarray(sin, dtype=np.float32)
    mask = np.asarray(mask, dtype=np.float32)

    variant = _detect_variant(mask)
    nc = _get_program(variant)
    in_maps = make_in_maps(x, wq, wk, wv, wo, cos, sin, mask, variant)

    global _last_in_maps
    _last_in_maps = in_maps

    res = run_bass_kernel_spmd(nc, in_maps, core_ids=list(range(NCORES)))

    out = np.empty((B, S, D), dtype=np.float32)
    gpb = NCORES // B
    for b in range(B):
        acc = np.zeros((S, D), dtype=np.float64)
        for g in range(gpb):
            acc += res.results[b * gpb + g]["y"].astype(np.float64)
        out[b] = acc.astype(np.float32)
    return out
